# revision 22
# baseline (speedup 1.0000x reference)
"""Trainium2 Bass kernel for nn_NetworksPlusCircuit.

Computation: y[b] = circuit(sigmoid(x[b] @ Ws + bs)) for x [1048576, 64].

Circuit simplification: f(i)=1 for i>=8, so only labelling columns 1..7
matter and the SDD collapses to

    f7 = l7 + 1
    f6 = l6*l7 + 1
    f5 = l5*(f6-f7) + f7
    f4 = l4*(f5-f6) + f6
    f3 = l3*f4 + f5
    f2 = l2*(f3-f4) + f4
    f1 = l1*(f2-f3) + f3

Sharding: pure data parallel over batch across 8 cores. Host ships x as
bf16 xT2 [128, 65536] (two 65536-batch halves stacked on the partition dim,
d-major) — 16 MB/core, half the f32 traffic; the kernel is HBM-read-bound.

HYBRID matmul dataflow. The kernel is simultaneously limited by the HBM
stream (~47-53 us) and the PE instruction stream, so the two halves of the
data use different matmul directions:

* Pairs 0..7 (xT2 cols [0, 32768), first 8 MB) — "wide" direction: the
  stationary operand is the block-diagonal weight w2 [128, 16] and X streams
  as the 512-col moving operand (8 exec-bound matmuls per MB — few PE
  instructions). Output is literal-on-partition, so the scalar engine
  applies sigmoid, the DVE does a 32x32 block transpose, and the circuit
  runs on strided literal views. All that DVE work overlaps the stream.
  Index map (v1): psum pair p covers cols [4096p, 4096p+4096), chunk
  n = 2g+e is cols [512n, 512n+512); literal j of batch
  (65536h + 16384t + 4096v + 1024g + 512e + 32c + r) lands in
  H_t[32g + r, 1024v + 512e + 32c + 8h + j]; F_t[32g+r, 64v+32e+2c+h].
* Fills 8..15 (cols [32768, 65536), last 8 MB) — "tall" direction: each
  [128, 128] slice of xT2 is the stationary operand and w2 the 16-col
  moving operand. Output is batch-on-partition: no transpose, contiguous
  circuit, so the post-stream tail is short. Literal j of batch
  (65536h + 4096f + 128i + m) sits, within a circuit group starting at
  fill f0 with plane width W = 64*nf, at S[m, W*j + 64*(f-f0) + 32h + i];
  output Y[m, 64*(f-8) + 32h + i].

The wide direction finishes its PE work early; the tall direction's pairs
then track the DMA stream, and the kernel ends right after the last 1 MB
lands plus a half-width contiguous circuit chain.
"""

import sys

for _p in ("/opt/trn_rl_repo",):
    if _p not in sys.path:
        sys.path.insert(0, _p)

import numpy as np
import ml_dtypes

N_CORES = 8
B_TOTAL = 1048576
D = 64
BC = B_TOTAL // N_CORES      # 131072 batch per core
HALF = BC // 2               # 65536 xT2 cols (batch-per-half)
NF = 16                      # 1 MB column blocks per core
NOLD = 8                     # blocks 0..7: wide direction (2 H-tiles)
FW = 4096                    # X cols per block; [128, 4096] bf16 = 1 MB
TPF = 32                     # stationary tiles per fill (tall direction)
# tall-direction circuit groups (first_fill, n_fills), shrinking toward the
# end so the final (serial, unoverlapped) circuit chain is as short as
# possible
NGROUPS = [(8, 4), (12, 2), (14, 1), (15, 1)]


def _split_multiwait_instructions(nc, mybir):
    """This walrus build accepts at most one sync wait per instruction.
    Split any multi-wait instruction into single-wait NoOps on the same
    engine ahead of it (engines execute their queue in order, so semantics
    are unchanged)."""
    n_split = 0
    for fn in nc.m.functions:
        for blk in fn.blocks:
            insts = blk.instructions
            if not any(
                i.sync_info is not None and len(i.sync_info.on_wait) > 1
                for i in insts
            ):
                continue
            out = []
            for inst in insts:
                si = inst.sync_info
                if si is not None and len(si.on_wait) > 1:
                    waits = list(si.on_wait)
                    for k, w in enumerate(waits[:-1]):
                        nop = mybir.InstNoOp(
                            name=f"{inst.name}-sw{k}",
                            engine=inst.engine,
                            ins=[],
                            outs=[],
                            sync_info=mybir.SyncInfo(on_wait=[w], on_update=[]),
                        )
                        out.append(nop)
                        n_split += 1
                    inst.sync_info = mybir.SyncInfo(
                        on_wait=[waits[-1]], on_update=list(si.on_update)
                    )
                out.append(inst)
            blk.instructions = out
    return n_split


def build_program(with_bias=False):
    import concourse.bass as bass
    import concourse.mybir as mybir
    from concourse import tile
    from contextlib import ExitStack

    f32 = mybir.dt.float32
    bf16 = mybir.dt.bfloat16
    SIG = mybir.ActivationFunctionType.Sigmoid
    A = mybir.AluOpType
    nc = bass.Bass("TRN2")
    xT2 = nc.dram_tensor("xT2", [128, HALF], bf16, kind="ExternalInput")
    w2 = nc.dram_tensor("w2", [128, 16], bf16, kind="ExternalInput")
    if with_bias:
        b2 = nc.dram_tensor("b2", [128, 1], f32, kind="ExternalInput")
        ones2 = nc.dram_tensor("ones2", [128, 128], bf16, kind="ExternalInput")
        bias2 = nc.dram_tensor("bias2", [128, 512], bf16, kind="ExternalInput")
    y_old = nc.dram_tensor("y_old", [2, 128, 256], f32, kind="ExternalOutput")
    y_new = nc.dram_tensor("y_new", [128, 512], f32, kind="ExternalOutput")

    with tile.TileContext(nc) as tc:
        with ExitStack() as ctx:
            wpool = ctx.enter_context(tc.tile_pool(name="wpool", bufs=1))
            xpool = ctx.enter_context(tc.tile_pool(name="xpool", bufs=6))
            xspool = ctx.enter_context(tc.tile_pool(name="xspool", bufs=4))
            spool = ctx.enter_context(tc.tile_pool(name="spool", bufs=2))
            hpool = ctx.enter_context(tc.tile_pool(name="hpool", bufs=2))
            cpool = ctx.enter_context(tc.tile_pool(name="cpool", bufs=1))
            opool = ctx.enter_context(
                tc.tile_pool(name="opool", bufs=2, space="PSUM")
            )
            ppool = ctx.enter_context(
                tc.tile_pool(name="ppool", bufs=4, space="PSUM")
            )

            wt = wpool.tile([128, 16], bf16)
            nc.sync.dma_start(wt[:], w2[:, :])
            if with_bias:
                bt = wpool.tile([128, 1], f32)
                nc.sync.dma_start(bt[:], b2[:, :])
                onest = wpool.tile([128, 128], bf16)
                nc.sync.dma_start(onest[:], ones2[:, :])
                biast = wpool.tile([128, 512], bf16)
                nc.sync.dma_start(biast[:], bias2[:, :])
            # Prime the sigmoid ACT table during the DMA ramp so the first
            # real activation doesn't pay the table load.
            warm = wpool.tile([128, 16], f32)
            nc.scalar.activation(warm[:], wt[:], SIG)

            # Persistent output accumulator for the tall region; chunked
            # stores (>=512 B partition rows) go out as regions complete.
            Y = wpool.tile([128, 512], f32)

            # X segmentation: dependency granularity is the whole tile, so
            # the ramp block and the last two blocks use smaller tiles.
            def x_tiles_for(f):
                if f == 0:
                    return [(0, 512), (512, 512), (1024, 1024), (2048, 2048)]
                if f < NOLD:
                    # wide direction is exec-bound on the PE; 512 KB tiles
                    # keep its matmuls tracking the stream closely
                    return [(f * FW, 2048), (f * FW + 2048, 2048)]
                if f >= NF - 2:
                    return [(f * FW + k * 1024, 1024) for k in range(4)]
                return [(f * FW, FW)]

            # ring depth per tile size, chosen so no X ring ever makes the
            # DMA stream wait on a lagging consumer
            XBUFS = {512: 2, 1024: 8, 2048: 4, 4096: 6}

            def load_x(f):
                segs = []
                for col0, cw in x_tiles_for(f):
                    pool = xpool if cw == FW else xspool
                    X = pool.tile(
                        [128, cw], bf16, name=f"X{cw}", tag=f"X{cw}",
                        bufs=XBUFS[cw])
                    nc.sync.dma_start(X[:], xT2[:, col0:col0 + cw])
                    segs.append((X, col0, cw))
                return segs

            def block512(segs, n):
                """(tile, local offset) of 512-col chunk n of this block."""
                want = segs[0][1] // FW * FW + 512 * n
                for X, col0, cw in segs:
                    if col0 <= want < col0 + cw:
                        return X, want - col0
                raise AssertionError("chunk not covered")

            # ---- wide direction: pairs 0..7, literal-on-partition ----
            def circuit_strided(H, t_idx):
                H3 = H.rearrange("p (b q) -> p b q", q=32)
                l = lambda j: H3[:, :, j:16:8]  # noqa: E731  [128, 128, 2]

                def t(name):
                    # same shape as the W=256 contiguous-circuit temps; the
                    # two circuits never overlap in DVE program order, so
                    # they share rings to save SBUF
                    t_ = cpool.tile(
                        [128, 256], f32, name=f"{name}_256", tag=f"{name}_256")
                    return t_.rearrange("p (b q) -> p b q", q=2)

                f7 = t("f7")
                nc.vector.tensor_scalar_add(f7, l(6), 1.0)
                pr = t("pr")
                nc.vector.tensor_mul(pr, l(5), l(6))
                f6 = t("f6")
                nc.vector.tensor_scalar_add(f6, pr, 1.0)
                e1 = t("e1")
                nc.vector.scalar_tensor_tensor(
                    e1, l(5), -1.0, l(6), A.add, A.mult)
                p2 = t("p2")
                nc.vector.tensor_mul(p2, l(4), e1)
                f5 = t("f5")
                nc.vector.tensor_add(f5, p2, f7)
                e2 = t("e2")
                nc.vector.scalar_tensor_tensor(
                    e2, l(4), -1.0, e1, A.add, A.mult)
                p3 = t("p3")
                nc.vector.tensor_mul(p3, l(3), e2)
                f4 = t("f4")
                nc.vector.tensor_add(f4, p3, f6)
                t3 = t("t3")
                nc.vector.scalar_tensor_tensor(
                    t3, l(2), -1.0, f4, A.add, A.mult)
                d3 = t("d3")
                nc.vector.tensor_add(d3, t3, f5)
                f3 = t("f3")
                nc.vector.tensor_add(f3, d3, f4)
                d4 = t("d4")
                nc.vector.scalar_tensor_tensor(
                    d4, l(1), -1.0, d3, A.add, A.mult)
                p6 = t("p6")
                nc.vector.tensor_mul(p6, l(0), d4)
                F = cpool.tile([128, 256], f32, name="F_o", tag="F_o")
                F3 = F.rearrange("p (b q) -> p b q", q=2)
                nc.vector.tensor_add(F3, p6, f3)
                nc.scalar.dma_start(y_old[t_idx], F[:])

            H = None
            for p in range(NOLD):
                t_idx, v = p // 4, p % 4
                segs = load_x(p)
                if v == 0:
                    H = hpool.tile([128, 4096], f32, name="H", tag="H")
                ps = opool.tile([128, 1024], f32)
                for g in range(4):
                    for e in range(2):
                        X, xo = block512(segs, 2 * g + e)
                        nc.tensor.matmul(
                            ps[32 * g:32 * g + 16, 512 * e:512 * (e + 1)],
                            wt[:, :],
                            X[:, xo:xo + 512],
                            start=True,
                            stop=True,
                            tile_position=(0, 32 * g),
                        )
                S = spool.tile([128, 1024], f32, name="So", tag="So")
                if with_bias:
                    nc.scalar.activation(
                        S[:], ps[:], SIG, bias=bt[:, 0:1], scale=1.0)
                else:
                    nc.scalar.activation(S[:], ps[:], SIG)
                nc.vector.transpose(H[:, 1024 * v:1024 * (v + 1)], S[:])
                if v == 3:
                    circuit_strided(H, t_idx)

            # ---- tall direction: fills 8..15, batch-on-partition ----
            def circuit_contig(S, W, ydst):
                # Same circuit, refactored for a short chain via
                #   f6-f7 = l7*(l6-1) =: e1    f5-f6 = e1*(l5-1) =: e2
                #   f3-f4 = (l3-1)*f4+f5 =: d3   f2-f3 = d3*(l2-1) =: d4
                #   f3 = d3+f4                 f1 = l1*d4 + f3
                l = lambda j: S[:, W * j:W * (j + 1)]  # noqa: E731

                def t(name):
                    nm = f"{name}_{W}"
                    return cpool.tile([128, W], f32, name=nm, tag=nm)

                f7 = t("f7")
                nc.vector.tensor_scalar_add(f7, l(6), 1.0)
                pr = t("pr")
                nc.vector.tensor_mul(pr, l(5), l(6))
                f6 = t("f6")
                nc.vector.tensor_scalar_add(f6, pr, 1.0)
                e1 = t("e1")
                nc.vector.scalar_tensor_tensor(
                    e1, l(5), -1.0, l(6), A.add, A.mult)
                p2 = t("p2")
                nc.vector.tensor_mul(p2, l(4), e1)
                f5 = t("f5")
                nc.vector.tensor_add(f5, p2, f7)
                e2 = t("e2")
                nc.vector.scalar_tensor_tensor(
                    e2, l(4), -1.0, e1, A.add, A.mult)
                p3 = t("p3")
                nc.vector.tensor_mul(p3, l(3), e2)
                f4 = t("f4")
                nc.vector.tensor_add(f4, p3, f6)
                t3 = t("t3")
                nc.vector.scalar_tensor_tensor(
                    t3, l(2), -1.0, f4, A.add, A.mult)
                d3 = t("d3")
                nc.vector.tensor_add(d3, t3, f5)
                f3 = t("f3")
                nc.vector.tensor_add(f3, d3, f4)
                d4 = t("d4")
                nc.vector.scalar_tensor_tensor(
                    d4, l(1), -1.0, d3, A.add, A.mult)
                p6 = t("p6")
                nc.vector.tensor_mul(p6, l(0), d4)
                nc.vector.tensor_add(ydst, p6, f3)

            for gi, (f0, nf) in enumerate(NGROUPS):
                W = 64 * nf
                S = spool.tile([128, 7 * W], f32, name=f"S{nf}", tag=f"S{nf}")
                for f in range(f0, f0 + nf):
                    segs = load_x(f)
                    ps = ppool.tile([128, 512], f32)
                    if with_bias:
                        nc.tensor.matmul(
                            ps[:, :], onest[:, :], biast[:, :],
                            start=True, stop=False, skip_group_check=True,
                        )
                    i = 0
                    for X, _, cw in segs:
                        for xo in range(0, cw, 128):
                            nc.tensor.matmul(
                                ps[:, 16 * i:16 * i + 16],
                                X[:, xo:xo + 128],
                                wt[:, :],
                                start=not with_bias,
                                stop=True,
                                skip_group_check=with_bias,
                            )
                            i += 1

                    # sigmoid + de-interleave: literal j of tile i -> plane j
                    psv = ps.rearrange("p (i s) -> p s i", s=16)
                    Sv = S.rearrange("p (s u) -> p s u", u=W)
                    o = 64 * (f - f0)
                    nc.scalar.activation(
                        Sv[:, :, o:o + 32], psv[:, 0:7, :], SIG)
                    nc.scalar.activation(
                        Sv[:, :, o + 32:o + 64], psv[:, 8:15, :], SIG)

                yo = 64 * (f0 - 8)
                circuit_contig(S, W, Y[:, yo:yo + W])
                # chunked output stores, each with >=512 B partition rows
                store = {0: (0, 256), 1: (256, 128), 3: (384, 128)}.get(gi)
                if store is not None:
                    o0, ow = store
                    nc.scalar.dma_start(
                        y_new[:, o0:o0 + ow], Y[:, o0:o0 + ow])

    import concourse.mybir as _mybir

    _split_multiwait_instructions(nc, _mybir)
    return nc


def _prep_inputs(x, Ws, bs):
    """Host-side shard + layout prep. Returns (per-core input maps, bias?)."""
    x = np.asarray(x, dtype=np.float32)
    Ws = np.asarray(Ws, dtype=np.float32)
    bs = np.asarray(bs, dtype=np.float32)

    W7 = np.zeros((64, 7), np.float32)
    b7 = np.zeros(7, np.float32)
    for j in range(7):
        W7[:, j] = Ws[j // 4, :, j % 4]
        b7[j] = bs[j // 4, j % 4]
    W2 = np.zeros((128, 16), np.float32)
    W2[0:64, 0:7] = W7
    W2[64:128, 8:15] = W7
    W2 = W2.astype(ml_dtypes.bfloat16)

    with_bias = bool(np.any(b7 != 0.0))
    extra = {}
    if with_bias:
        B2 = np.zeros((128, 1), np.float32)
        for g in range(4):
            for h in range(2):
                B2[32 * g + 8 * h:32 * g + 8 * h + 7, 0] = b7
        ones2 = np.ones((128, 128), ml_dtypes.bfloat16)
        bias2 = np.zeros((128, 512), np.float32)
        for s in range(7):
            bias2[:, s::16] = b7[s] / 128.0
            bias2[:, 8 + s::16] = b7[s] / 128.0
        extra = {"b2": B2, "ones2": ones2,
                 "bias2": bias2.astype(ml_dtypes.bfloat16)}

    in_maps = []
    for c in range(N_CORES):
        xc = x[c * BC:(c + 1) * BC]
        xT2 = np.ascontiguousarray(
            xc.reshape(2, HALF, D).transpose(0, 2, 1).reshape(128, HALF)
        ).astype(ml_dtypes.bfloat16)
        in_maps.append({"xT2": xT2, "w2": W2, **extra})
    return in_maps, with_bias


def _gather_output(results):
    """Invert the device layouts; see module docstring for the index maps."""
    outs = []
    for c in range(N_CORES):
        yo = np.asarray(results[c]["y_old"], dtype=np.float32)
        yn = np.asarray(results[c]["y_new"], dtype=np.float32)
        # wide region: batches [0, 32768) of each half
        po = (
            yo.reshape(2, 4, 32, 4, 2, 16, 2)   # t g r v e c h
            .transpose(6, 0, 3, 1, 4, 5, 2)     # h t v g e c r
            .reshape(2, 32768)
        )
        # tall region: batches [32768, 65536) of each half
        pn = (
            yn.reshape(128, 8, 2, 32)           # m f' h i
            .transpose(2, 1, 3, 0)              # h f' i m
            .reshape(2, 32768)
        )
        yc = np.empty(BC, np.float32)
        yc[0:32768] = po[0]
        yc[32768:65536] = pn[0]
        yc[65536:98304] = po[1]
        yc[98304:131072] = pn[1]
        outs.append(yc)
    return np.concatenate(outs).astype(np.float32)


def run(inputs, trace=False, **run_kwargs):
    """Build, execute on 8 cores, and gather. Returns (y, BassKernelResults)."""
    from concourse.bass_utils import run_bass_kernel_spmd

    in_maps, with_bias = _prep_inputs(inputs["x"], inputs["Ws"], inputs["bs"])
    nc = build_program(with_bias=with_bias)
    res = run_bass_kernel_spmd(
        nc, in_maps, core_ids=list(range(N_CORES)), trace=trace, **run_kwargs
    )
    return _gather_output(res.results), res


def kernel(x, Ws, bs):
    y, _ = run({"x": x, "Ws": Ws, "bs": bs})
    return y


if __name__ == "__main__":
    rng = np.random.default_rng(0)
    x = rng.standard_normal((B_TOTAL, D), dtype=np.float32)
    Ws = (rng.standard_normal((4, 64, 4)) * 0.1).astype(np.float32)
    bs = np.zeros((4, 4), np.float32)
    y = kernel(x, Ws, bs)
    print("kernel ran, y:", y.shape, y.dtype, y[:4])


# revision 23
# speedup vs baseline: 1.0384x; 1.0384x over previous
"""Trainium2 Bass kernel for nn_NetworksPlusCircuit.

Computation: y[b] = circuit(sigmoid(x[b] @ Ws + bs)) for x [1048576, 64].

Circuit simplification (see git history / reference): f(i)=1 for i>=8, so only
labelling columns 1..7 matter and the SDD collapses to

    f7 = l7 + 1
    f6 = l6*l7 + 1
    f5 = l5*(f6-f7) + f7
    f4 = l4*(f5-f6) + f6
    f3 = l3*f4 + f5
    f2 = l2*(f3-f4) + f4
    f1 = l1*(f2-f3) + f3

Sharding: pure data parallel over batch across 8 cores.

Device dataflow (v2 — batch-on-partition, bf16 stream):
  * Host ships x as bf16 xT2 [128, 65536]: two 65536-batch halves stacked on
    the partition dim, d-major (partition 64h+d, free = batch-within-half).
    16 MB/core instead of 32 MB — the kernel is HBM-read-bound, so bf16
    halves the roofline. (bf16 end-to-end max rel err ~4e-3, gate is 2e-2.)
  * Matmul direction is flipped vs v1: the STATIONARY operand is a [128, 128]
    slice of xT2 (128 d-rows x 128 batch-cols, fast weight load at 2 bf16
    cols/cycle) and the MOVING operand is the tiny block-diagonal weight
    w2 [128, 16] (rows 0:64 -> cols 0:7 = W7 for the half-0 batch, rows
    64:128 -> cols 8:15 = W7 for the half-1 batch). Output lands [128 batch
    partitions, 16 literal slots] in PSUM — batch is already on partitions,
    so NO on-chip transpose is needed at all.
  * 32 matmuls fill one PSUM bank [128, 512] = 32 tiles x 16 slots. The
    scalar engine applies sigmoid while DE-INTERLEAVING: two strided
    activations per bank scatter literal j of tile i to contiguous
    per-literal planes in SBUF. After 4 banks (a "quarter"), each literal
    occupies a contiguous [128, 256] plane, so the 17-op circuit runs as
    cheap contiguous DVE ops. f32 planes + f32 circuit preserve precision.
  * Output F [128, 256] per quarter -> y [4, 128, 256]; host inverts the
    layout permutation.

Per-core index map (core-local batch): stationary tile p covers xT2 cols
[128p, 128p+128); batch = 65536h + 128p + m (m = col within tile, h = half).
Fill f = p//32 (i = p%32). Within a circuit group starting at fill f0 with
plane width W = 64*nf, literal j of (h,i,m) sits at
S[m, W*j + 64*(f-f0) + 32*h + i]; the final output is stored flat as
y[m, 64*f + 32*h + i] = f1(batch = 65536h + 4096f + 128i + m).
"""

import sys

for _p in ("/opt/trn_rl_repo",):
    if _p not in sys.path:
        sys.path.insert(0, _p)

import numpy as np
import ml_dtypes

N_CORES = 8
B_TOTAL = 1048576
D = 64
BC = B_TOTAL // N_CORES      # 131072 batch per core
HALF = BC // 2               # 65536 xT2 cols (batch-per-half)
NF = 16                      # psum bank fills per core
TPF = 32                     # stationary tiles (matmuls) per fill
FW = 4096                    # X cols per fill; [128, 4096] bf16 = 1 MB
# circuit groups (first_fill, n_fills): shrinking toward the end so the
# final (serial, unoverlapped) circuit chain is as short as possible
GROUPS = [(0, 4), (4, 4), (8, 4), (12, 2), (14, 1), (15, 1)]


def _split_multiwait_instructions(nc, mybir):
    """This walrus build accepts at most one sync wait per instruction.
    Split any multi-wait instruction into single-wait NoOps on the same
    engine ahead of it (engines execute their queue in order, so semantics
    are unchanged)."""
    n_split = 0
    for fn in nc.m.functions:
        for blk in fn.blocks:
            insts = blk.instructions
            if not any(
                i.sync_info is not None and len(i.sync_info.on_wait) > 1
                for i in insts
            ):
                continue
            out = []
            for inst in insts:
                si = inst.sync_info
                if si is not None and len(si.on_wait) > 1:
                    waits = list(si.on_wait)
                    for k, w in enumerate(waits[:-1]):
                        nop = mybir.InstNoOp(
                            name=f"{inst.name}-sw{k}",
                            engine=inst.engine,
                            ins=[],
                            outs=[],
                            sync_info=mybir.SyncInfo(on_wait=[w], on_update=[]),
                        )
                        out.append(nop)
                        n_split += 1
                    inst.sync_info = mybir.SyncInfo(
                        on_wait=[waits[-1]], on_update=list(si.on_update)
                    )
                out.append(inst)
            blk.instructions = out
    return n_split


def build_program(with_bias=False):
    import concourse.bass as bass
    import concourse.mybir as mybir
    from concourse import tile
    from contextlib import ExitStack

    f32 = mybir.dt.float32
    bf16 = mybir.dt.bfloat16
    SIG = mybir.ActivationFunctionType.Sigmoid
    nc = bass.Bass("TRN2")
    xT2 = nc.dram_tensor("xT2", [128, HALF], bf16, kind="ExternalInput")
    w2 = nc.dram_tensor("w2", [128, 16], bf16, kind="ExternalInput")
    if with_bias:
        ones2 = nc.dram_tensor("ones2", [128, 128], bf16, kind="ExternalInput")
        bias2 = nc.dram_tensor("bias2", [128, 512], bf16, kind="ExternalInput")
    y = nc.dram_tensor("y", [128, 1024], f32, kind="ExternalOutput")

    with tile.TileContext(nc) as tc:
        with ExitStack() as ctx:
            wpool = ctx.enter_context(tc.tile_pool(name="wpool", bufs=1))
            xpool = ctx.enter_context(tc.tile_pool(name="xpool", bufs=8))
            spool = ctx.enter_context(tc.tile_pool(name="spool", bufs=3))
            cpool = ctx.enter_context(tc.tile_pool(name="cpool", bufs=1))
            ppool = ctx.enter_context(
                tc.tile_pool(name="ppool", bufs=8, space="PSUM")
            )

            wt = wpool.tile([128, 16], bf16)
            nc.sync.dma_start(wt[:], w2[:, :])
            if with_bias:
                onest = wpool.tile([128, 128], bf16)
                nc.sync.dma_start(onest[:], ones2[:, :])
                biast = wpool.tile([128, 512], bf16)
                nc.sync.dma_start(biast[:], bias2[:, :])
            # Prime the sigmoid ACT table during the DMA ramp so the first
            # real activation doesn't pay the table load.
            warm = wpool.tile([128, 16], f32)
            nc.scalar.activation(warm[:], wt[:], SIG)

            # Persistent output accumulator: the circuit writes f1 straight
            # into Y; chunked stores (>=512 B per partition row) go out as
            # regions complete so only a 64 KB store trails the last group.
            Y = wpool.tile([128, 1024], f32)

            def circuit(S, W, ydst):
                # Circuit, refactored for a short chain using the identities
                #   f6-f7 = l7*(l6-1) =: e1      f5-f6 = e1*(l5-1) =: e2
                #   f3-f4 = (l3-1)*f4 + f5 =: d3      f2-f3 = d3*(l2-1) =: d4
                #   f3 = d3 + f4                f1 = l1*d4 + f3
                # scalar_tensor_tensor fuses each (l-1)*t pair: 15 ops total.
                l = lambda j: S[:, W * j:W * (j + 1)]  # noqa: E731
                A = mybir.AluOpType

                def t(name):
                    nm = f"{name}_{W}"
                    return cpool.tile([128, W], f32, name=nm, tag=nm)

                f7 = t("f7")
                nc.vector.tensor_scalar_add(f7, l(6), 1.0)
                pr = t("pr")
                nc.vector.tensor_mul(pr, l(5), l(6))
                f6 = t("f6")
                nc.vector.tensor_scalar_add(f6, pr, 1.0)
                e1 = t("e1")
                nc.vector.scalar_tensor_tensor(
                    e1, l(5), -1.0, l(6), A.add, A.mult)
                p2 = t("p2")
                nc.vector.tensor_mul(p2, l(4), e1)
                f5 = t("f5")
                nc.vector.tensor_add(f5, p2, f7)
                e2 = t("e2")
                nc.vector.scalar_tensor_tensor(
                    e2, l(4), -1.0, e1, A.add, A.mult)
                p3 = t("p3")
                nc.vector.tensor_mul(p3, l(3), e2)
                f4 = t("f4")
                nc.vector.tensor_add(f4, p3, f6)
                t3 = t("t3")
                nc.vector.scalar_tensor_tensor(
                    t3, l(2), -1.0, f4, A.add, A.mult)
                d3 = t("d3")
                nc.vector.tensor_add(d3, t3, f5)
                f3 = t("f3")
                nc.vector.tensor_add(f3, d3, f4)
                d4 = t("d4")
                nc.vector.scalar_tensor_tensor(
                    d4, l(1), -1.0, d3, A.add, A.mult)
                p6 = t("p6")
                nc.vector.tensor_mul(p6, l(0), d4)
                nc.vector.tensor_add(ydst, p6, f3)

            # X tile segmentation: dependency granularity is the whole tile,
            # so the first fill (pipeline ramp) and the last two fills (tail)
            # use smaller tiles than the steady-state 1 MB.
            def x_tiles_for(f):
                if f == 0:
                    # extra-fine ramp: the PE stream is dispatch-bound and
                    # paces the kernel end, so start it as early as possible
                    return [(0, 512), (512, 512), (1024, 1024), (2048, 2048)]
                if f >= NF - 2:
                    return [(f * FW + k * 1024, 1024) for k in range(4)]
                return [(f * FW, FW)]

            for gi, (f0, nf) in enumerate(GROUPS):
                W = 64 * nf
                S = spool.tile([128, 7 * W], f32, name=f"S{nf}", tag=f"S{nf}")
                for f in range(f0, f0 + nf):
                    segs = []
                    for col0, cw in x_tiles_for(f):
                        X = xpool.tile(
                            [128, cw], bf16, name=f"X{cw}", tag=f"X{cw}")
                        nc.sync.dma_start(X[:], xT2[:, col0:col0 + cw])
                        segs.append((X, cw))

                    ps = ppool.tile([128, 512], f32)
                    if with_bias:
                        nc.tensor.matmul(
                            ps[:, :], onest[:, :], biast[:, :],
                            start=True, stop=False, skip_group_check=True,
                        )
                    i = 0
                    for X, cw in segs:
                        for xo in range(0, cw, 128):
                            nc.tensor.matmul(
                                ps[:, 16 * i:16 * i + 16],
                                X[:, xo:xo + 128],
                                wt[:, :],
                                start=not with_bias,
                                stop=True,
                                skip_group_check=with_bias,
                            )
                            i += 1

                    # sigmoid + de-interleave: literal j of tile i -> plane j
                    psv = ps.rearrange("p (i s) -> p s i", s=16)
                    Sv = S.rearrange("p (s u) -> p s u", u=W)
                    o = 64 * (f - f0)
                    nc.scalar.activation(
                        Sv[:, :, o:o + 32], psv[:, 0:7, :], SIG)
                    nc.scalar.activation(
                        Sv[:, :, o + 32:o + 64], psv[:, 8:15, :], SIG)

                circuit(S, W, Y[:, 64 * f0:64 * f0 + W])
                # chunked output stores, each with >=512 B partition rows
                store = {2: (0, 512), 3: (512, 256), 4: (768, 128),
                         5: (896, 128)}.get(gi)
                if store is not None:
                    o0, ow = store
                    nc.scalar.dma_start(y[:, o0:o0 + ow], Y[:, o0:o0 + ow])

    import concourse.mybir as _mybir

    _split_multiwait_instructions(nc, _mybir)
    return nc


def _prep_inputs(x, Ws, bs):
    """Host-side shard + layout prep. Returns (per-core input maps, bias?)."""
    x = np.asarray(x, dtype=np.float32)
    Ws = np.asarray(Ws, dtype=np.float32)
    bs = np.asarray(bs, dtype=np.float32)

    W7 = np.zeros((64, 7), np.float32)
    b7 = np.zeros(7, np.float32)
    for j in range(7):
        W7[:, j] = Ws[j // 4, :, j % 4]
        b7[j] = bs[j // 4, j % 4]
    W2 = np.zeros((128, 16), np.float32)
    W2[0:64, 0:7] = W7
    W2[64:128, 8:15] = W7
    W2 = W2.astype(ml_dtypes.bfloat16)

    with_bias = bool(np.any(b7 != 0.0))
    extra = {}
    if with_bias:
        ones2 = np.ones((128, 128), ml_dtypes.bfloat16)
        bias2 = np.zeros((128, 512), np.float32)
        for s in range(7):
            bias2[:, s::16] = b7[s] / 128.0
            bias2[:, 8 + s::16] = b7[s] / 128.0
        extra = {"ones2": ones2, "bias2": bias2.astype(ml_dtypes.bfloat16)}

    in_maps = []
    for c in range(N_CORES):
        xc = x[c * BC:(c + 1) * BC]
        xT2 = np.ascontiguousarray(
            xc.reshape(2, HALF, D).transpose(0, 2, 1).reshape(128, HALF)
        ).astype(ml_dtypes.bfloat16)
        in_maps.append({"xT2": xT2, "w2": W2, **extra})
    return in_maps, with_bias


def _gather_output(results):
    """Invert the device layout; see module docstring for the index map."""
    outs = []
    for c in range(N_CORES):
        yraw = np.asarray(results[c]["y"], dtype=np.float32)
        yc = (
            yraw.reshape(128, NF, 2, 32)       # m f h i
            .transpose(2, 1, 3, 0)             # h f i m
            .reshape(BC)
        )
        outs.append(yc)
    return np.concatenate(outs).astype(np.float32)


def run(inputs, trace=False, **run_kwargs):
    """Build, execute on 8 cores, and gather. Returns (y, BassKernelResults)."""
    from concourse.bass_utils import run_bass_kernel_spmd

    in_maps, with_bias = _prep_inputs(inputs["x"], inputs["Ws"], inputs["bs"])
    nc = build_program(with_bias=with_bias)
    res = run_bass_kernel_spmd(
        nc, in_maps, core_ids=list(range(N_CORES)), trace=trace, **run_kwargs
    )
    return _gather_output(res.results), res


def kernel(x, Ws, bs):
    y, _ = run({"x": x, "Ws": Ws, "bs": bs})
    return y


if __name__ == "__main__":
    rng = np.random.default_rng(0)
    x = rng.standard_normal((B_TOTAL, D), dtype=np.float32)
    Ws = (rng.standard_normal((4, 64, 4)) * 0.1).astype(np.float32)
    bs = np.zeros((4, 4), np.float32)
    y = kernel(x, Ws, bs)
    print("kernel ran, y:", y.shape, y.dtype, y[:4])
